# revision 7
# baseline (speedup 1.0000x reference)
"""Trainium2 Bass kernel for nn_Decoder (single-step attention LSTM decoder).

Sharding (8 cores):
  - Attention: data-parallel over batch (8 batches/core); encoder outputs
    pre-transposed per batch so the energy matmul contracts over d on
    partitions. Context shards are AllGather'd.
  - Embedding: table column-sharded (E/8 cols per core); every core gathers
    all 64 tokens from its slice; transposed shards AllGather'd into embT.
  - LSTM: tensor-parallel over hidden units (128 units/core; the matching
    i/f/g/o rows of w_ih/w_hh). New h shards transposed + AllGather'd.
  - fc_out/logits: column-parallel over vocab (4000 rows of fc_w per core),
    fed phase-by-phase (embedded, context, h1) so weight streaming overlaps
    the attention/LSTM critical path.
"""
import numpy as np

import concourse.bass as bass
import concourse.mybir as mybir
import concourse.tile as tile
from concourse.bass_utils import run_bass_kernel_spmd
from concourse.masks import make_identity

FP = mybir.dt.float32
I32 = mybir.dt.int32
AX = mybir.AxisListType
OP = mybir.AluOpType
ACTF = mybir.ActivationFunctionType

NC = 8
V, E, H2, H, B, S = 32000, 512, 1024, 512, 64, 128
VS = V // NC      # 4000 vocab rows per core
BL = B // NC      # 8 batches per core
U = H2 // NC      # 128 hidden units per core
ES = E // NC      # 64 embedding cols per core
FCK = H2 + H2 + E  # 2560 fc contraction
KE = H2 // 128     # 8 k-tiles per 1024
MT = H // 128      # 4 h-tiles of attention inner dim


def split_multiwait(nc, max_waits=1):
    """This container's walrus rejects >1 sync-wait per instruction. Hoist
    extra waits onto single-wait NoOps inserted before the instruction on
    the same engine (earlier program order preserves semantics)."""
    ctr = 0
    for fn in nc.m.functions:
        for bb in fn.blocks:
            insts = list(bb.instructions)
            out = []
            changed = False
            for inst in insts:
                si = getattr(inst, "sync_info", None)
                waits = list(si.on_wait) if (si is not None and si.on_wait) else []
                if len(waits) > max_waits:
                    changed = True
                    for w in waits[:-max_waits]:
                        ctr += 1
                        out.append(mybir.InstNoOp(
                            name=f"waitsplit_{ctr}_{inst.name}",
                            engine=inst.engine,
                            sync_info=mybir.SyncInfo(on_wait=[w], on_update=[]),
                            bass_nofuse=True,
                        ))
                    si.on_wait = waits[-max_waits:]
                out.append(inst)
            if changed:
                try:
                    bb.instructions[:] = out
                except Exception:
                    bb.instructions = out
    return ctr


def build():
    nc = bass.Bass(num_devices=NC)

    def inp(name, shape, dtype=FP):
        return nc.declare_dram_parameter(name, list(shape), dtype, isOutput=False)

    def outp(name, shape, dtype=FP):
        return nc.declare_dram_parameter(name, list(shape), dtype, isOutput=True)

    tokens = inp("tokens", [B], I32)
    emb_sh = inp("emb_sh", [V, ES])          # emb_table[:, m*ES:(m+1)*ES]
    h0T = inp("h0T", [H2, B])                # hidden[0].T
    h1T = inp("h1T", [H2, B])                # hidden[1].T
    h1Tl = inp("h1Tl", [H2, BL])             # hidden[1].T local batch cols
    c0s = inp("c0s", [B, U])                 # cell[0][:, m*U:(m+1)*U]
    c1s = inp("c1s", [B, U])
    enc_p = inp("enc_p", [H2, BL * S])       # enc[d, (b, s)] local batches
    maskf = inp("maskf", [1, BL * S])        # float mask, (b, s)
    attn_wT = inp("attn_wT", [4 * H, H])     # attn_w.T (rows: h-half, enc-half)
    v_pk = inp("v_pk", [128, MT])            # v_w packed [128, 4], col = k-tile
    ab_pk = inp("ab_pk", [128, MT])          # attn_b packed [128, 4]
    wih0T = inp("wih0T", [E + H2, 4 * U])    # sharded+transposed LSTM weights
    whh0T = inp("whh0T", [H2, 4 * U])
    wih1T = inp("wih1T", [H2, 4 * U])
    whh1T = inp("whh1T", [H2, 4 * U])
    bias0 = inp("bias0", [1, 4 * U])         # (b_ih0+b_hh0) shard
    bias1 = inp("bias1", [1, 4 * U])
    fcwT = inp("fcwT", [FCK, VS])            # fc_w shard transposed
    fcb = inp("fcb", [1, VS])

    pred = outp("pred", [B, VS])
    h0n_o = outp("h0n", [B, U])
    c0n_o = outp("c0n", [B, U])
    h1n_o = outp("h1n", [B, U])
    c1n_o = outp("c1n", [B, U])
    attnw_o = outp("attnw", [BL, S])

    with tile.TileContext(nc) as tc:
        with (
            tc.tile_pool(name="const", bufs=1) as pc,
            tc.tile_pool(name="enc", bufs=KE) as pe,
            tc.tile_pool(name="wt", bufs=12) as pw,
            tc.tile_pool(name="fcw", bufs=6) as pf,
            tc.tile_pool(name="xk", bufs=1) as px,
            tc.tile_pool(name="energy", bufs=16) as pen,
            tc.tile_pool(name="work", bufs=2) as pk,
            tc.tile_pool(name="acc", bufs=1) as pa,
            tc.tile_pool(name="psfc", bufs=2, space="PSUM") as ps_fc,
            tc.tile_pool(name="pse", bufs=2, space="PSUM") as ps_e,
            tc.tile_pool(name="pssc", bufs=2, space="PSUM") as ps_sc,
            tc.tile_pool(name="psmisc", bufs=2, space="PSUM") as ps_m,
            tc.tile_pool(name="dram", bufs=1, space="DRAM") as dr,
        ):
            # ---------- constants / small loads ----------
            ident = pc.tile([B, B], FP)
            make_identity(nc, ident[:])
            ones = pc.tile([1, 128], FP)
            nc.vector.memset(ones[:], 1.0)
            tok_sb = pc.tile([B, 1], I32)
            nc.sync.dma_start(tok_sb[:, 0], tokens[:])
            v_sb = pc.tile([128, MT], FP)
            nc.sync.dma_start(v_sb[:], v_pk[:])
            ab_sb = pc.tile([128, MT], FP)
            nc.sync.dma_start(ab_sb[:], ab_pk[:])
            mask_sb = pc.tile([1, BL * S], FP)
            nc.sync.dma_start(mask_sb[:], maskf[:])
            b0_sb = pc.tile([1, 4 * U], FP)
            nc.sync.dma_start(b0_sb[:], bias0[:])
            b1_sb = pc.tile([1, 4 * U], FP)
            nc.sync.dma_start(b1_sb[:], bias1[:])

            # old hidden transposed, as k-tiles [128, B]
            h0T_sb = [px.tile([128, B], FP, tag=f"h0T{k}", name=f"h0T{k}") for k in range(KE)]
            h1T_sb = [px.tile([128, B], FP, tag=f"h1T{k}", name=f"h1T{k}") for k in range(KE)]
            h1l_sb = [px.tile([128, BL], FP, tag=f"h1l{k}", name=f"h1l{k}") for k in range(KE)]
            for k in range(KE):
                nc.sync.dma_start(h0T_sb[k][:], h0T[k * 128:(k + 1) * 128, :])
                nc.sync.dma_start(h1T_sb[k][:], h1T[k * 128:(k + 1) * 128, :])
                nc.sync.dma_start(h1l_sb[k][:], h1Tl[k * 128:(k + 1) * 128, :])

            # ---------- embedding: gather from column shard, AG transposed ----------
            emb_g = pk.tile([B, ES], FP, tag="embg")
            nc.gpsimd.indirect_dma_start(
                out=emb_g[:], out_offset=None, in_=emb_sh[:],
                in_offset=bass.IndirectOffsetOnAxis(ap=tok_sb[:, :1], axis=0),
            )
            embT_ps = ps_m.tile([ES, B], FP, tag="misc", space="PSUM")
            nc.tensor.transpose(embT_ps[:, :], emb_g[:], ident[:])
            embT_part = pk.tile([ES, B], FP, tag="embTp")
            nc.vector.tensor_copy(embT_part[:], embT_ps[:])
            emb_bnc = dr.tile([ES, B], FP)
            nc.sync.dma_start(emb_bnc[:], embT_part[:])
            emb_ag = dr.tile([E, B], FP)
            nc.gpsimd.collective_compute(
                "AllGather", OP.bypass, replica_groups=[list(range(NC))],
                ins=[emb_bnc.opt()], outs=[emb_ag.opt()],
            )
            embT_sb = [px.tile([128, B], FP, tag=f"embT{k}", name=f"embT{k}") for k in range(E // 128)]
            for k in range(E // 128):
                nc.sync.dma_start(embT_sb[k][:], emb_ag[k * 128:(k + 1) * 128, :])

            # ---------- attention inputs ----------
            enc_sb = [pe.tile([128, BL * S], FP, tag="enc", name="enc_t") for _ in range(KE)]
            for k in range(KE):
                nc.sync.dma_start(enc_sb[k][:], enc_p[k * 128:(k + 1) * 128, :])
            Wh = [pw.tile([128, H], FP, tag="wt", name="attw_t") for _ in range(KE)]
            for k in range(KE):
                nc.sync.dma_start(Wh[k][:], attn_wT[k * 128:(k + 1) * 128, :])
            We = [pw.tile([128, H], FP, tag="wt", name="attw_t") for _ in range(KE)]
            for k in range(KE):
                nc.sync.dma_start(We[k][:], attn_wT[H2 + k * 128:H2 + (k + 1) * 128, :])

            # h-part of energy: hb[mt] [128, BL] = (Wh.T @ h1T_local) + attn_b
            hb_sb = []
            for mt in range(MT):
                ps_hp = ps_m.tile([128, BL], FP, tag="misc", space="PSUM")
                for k in range(KE):
                    nc.tensor.matmul(
                        ps_hp[:, :], Wh[k][:, mt * 128:(mt + 1) * 128], h1l_sb[k][:],
                        start=(k == 0), stop=(k == KE - 1),
                    )
                hb = pk.tile([128, BL], FP, tag=f"hb{mt}")
                nc.vector.tensor_scalar_add(hb[:], ps_hp[:], ab_sb[:, mt:mt + 1])
                hb_sb.append(hb)

            penalty = pc.tile([1, BL * S], FP)
            nc.vector.tensor_scalar(
                out=penalty[:], in0=mask_sb[:], scalar1=1e10, scalar2=-1e10,
                op0=OP.mult, op1=OP.add,
            )
            attnw_sb = pc.tile([1, BL * S], FP)
            ctxT_loc = [pk.tile([128, BL], FP, tag=f"ctxl{dt}", name=f"ctxl{dt}") for dt in range(KE)]

            # energy + scores + softmax + context, per 4-batch group
            for bg in range(2):
                energy = [[None] * MT for _ in range(4)]
                for mt in range(MT):
                    ps_en = ps_e.tile([128, 512], FP, tag="e", space="PSUM")
                    for k in range(KE):
                        for bl in range(4):
                            b = bg * 4 + bl
                            nc.tensor.matmul(
                                ps_en[:, bl * 128:(bl + 1) * 128],
                                We[k][:, mt * 128:(mt + 1) * 128],
                                enc_sb[k][:, b * S:(b + 1) * S],
                                start=(k == 0 and bl == 0),
                                stop=(k == KE - 1 and bl == 3),
                            )
                    for bl in range(4):
                        b = bg * 4 + bl
                        e_sb = pen.tile([128, S], FP, tag="energy")
                        nc.scalar.activation(
                            e_sb[:], ps_en[:, bl * 128:(bl + 1) * 128],
                            ACTF.Tanh, bias=hb_sb[mt][:, b:b + 1],
                        )
                        energy[bl][mt] = e_sb

                ps_s = ps_sc.tile([1, 512], FP, tag="sc", space="PSUM")
                for bl in range(4):
                    for mt in range(MT):
                        nc.tensor.matmul(
                            ps_s[:, bl * 128:(bl + 1) * 128],
                            v_sb[:, mt:mt + 1], energy[bl][mt][:],
                            start=(mt == 0 and bl == 0),
                            stop=(mt == MT - 1 and bl == 3),
                        )
                for bl in range(4):
                    b = bg * 4 + bl
                    sl = slice(b * S, (b + 1) * S)
                    scm = pk.tile([1, S], FP, tag="scm")
                    nc.vector.tensor_tensor(
                        out=scm[:], in0=ps_s[:, bl * 128:(bl + 1) * 128],
                        in1=mask_sb[:, sl], op=OP.mult,
                    )
                    masked = pk.tile([1, S], FP, tag="masked")
                    nc.vector.tensor_tensor(
                        out=masked[:], in0=scm[:], in1=penalty[:, sl], op=OP.add,
                    )
                    nmx = pk.tile([1, 1], FP, tag="nmx")
                    nc.vector.tensor_reduce(
                        out=nmx[:], in_=masked[:], axis=AX.X, op=OP.max, negate=True,
                    )
                    erow = pk.tile([1, S], FP, tag="erow")
                    ssum = pk.tile([1, 1], FP, tag="ssum")
                    nc.scalar.activation(
                        erow[:], masked[:], ACTF.Exp, bias=nmx[:], accum_out=ssum[:],
                    )
                    rcp = pk.tile([1, 1], FP, tag="rcp")
                    nc.vector.reciprocal(rcp[:], ssum[:])
                    nc.vector.tensor_scalar_mul(attnw_sb[:, sl], erow[:], rcp[:])

                    # broadcast attn row across partitions via K=1 matmul
                    ps_ab = ps_m.tile([128, S], FP, tag="misc", space="PSUM")
                    nc.tensor.matmul(
                        ps_ab[:, :], ones[:, :], attnw_sb[:, sl],
                        start=True, stop=True,
                    )
                    for dt in range(KE):
                        scr = pk.tile([128, S], FP, tag="scr")
                        nc.vector.tensor_tensor(
                            out=scr[:], in0=enc_sb[dt][:, sl], in1=ps_ab[:],
                            op=OP.mult,
                        )
                        nc.vector.tensor_reduce(
                            out=ctxT_loc[dt][:, b:b + 1], in_=scr[:],
                            axis=AX.X, op=OP.add,
                        )
            nc.sync.dma_start(
                attnw_o[:].rearrange("(o b) s -> o (b s)", o=1), attnw_sb[:])

            # AllGather context: [H2, BL] shards -> [NC*H2, BL], then
            # interleaved reload into ctxT [128, B] k-tiles.
            ctx_bnc = dr.tile([H2, BL], FP)
            for dt in range(KE):
                nc.sync.dma_start(ctx_bnc[dt * 128:(dt + 1) * 128, :], ctxT_loc[dt][:])
            ctx_ag = dr.tile([NC * H2, BL], FP)
            nc.gpsimd.collective_compute(
                "AllGather", OP.bypass, replica_groups=[list(range(NC))],
                ins=[ctx_bnc.opt()], outs=[ctx_ag.opt()],
            )
            ctxT_sb = [px.tile([128, B], FP, tag=f"ctxT{k}", name=f"ctxT{k}") for k in range(KE)]
            ctx_r = ctx_ag[:].rearrange("(c dt p) bl -> dt p c bl", c=NC, dt=KE, p=128)
            for dt in range(KE):
                nc.sync.dma_start(
                    ctxT_sb[dt][:].rearrange("p (c bl) -> p c bl", c=NC), ctx_r[dt])

            # ---------- fc (vocab-sharded logits), phased ----------
            pred_acc = pa.tile([B, VS], FP)
            fck_lhs = {}
            for k in range(E // 128):
                fck_lhs[16 + k] = embT_sb[k]
            for k in range(KE):
                fck_lhs[8 + k] = ctxT_sb[k]

            def fc_subphase(kts, first, with_bias):
                for c in range(2):  # column half: fc cols [c*2000, c*2000+2000)
                    chunks = []
                    for kt in kts:
                        t = pf.tile([128, 2000], FP, tag="fcw")
                        nc.sync.dma_start(
                            t[:],
                            fcwT[kt * 128:(kt + 1) * 128, c * 2000:(c + 1) * 2000])
                        chunks.append(t)
                    for h in range(4):
                        nb = c * 4 + h
                        ps_p = ps_fc.tile([B, 500], FP, tag="fc", space="PSUM")
                        for j, kt in enumerate(kts):
                            nc.tensor.matmul(
                                ps_p[:, :], fck_lhs[kt][:],
                                chunks[j][:, h * 500:(h + 1) * 500],
                                start=(j == 0),
                                stop=(j == len(kts) - 1 and not with_bias),
                            )
                        dst = pred_acc[:, nb * 500:(nb + 1) * 500]
                        if with_bias:
                            fcb_t = pk.tile([1, 500], FP, tag="fcb")
                            nc.sync.dma_start(fcb_t[:], fcb[:, nb * 500:(nb + 1) * 500])
                            nc.tensor.matmul(
                                ps_p[:, :], ones[:, 0:B], fcb_t[:],
                                start=False, stop=True,
                            )
                        if first:
                            nc.vector.tensor_copy(dst, ps_p[:])
                        else:
                            nc.vector.tensor_add(out=dst, in0=dst, in1=ps_p[:])

            # phase 1: embedded part (+ bias) — overlaps attention
            fc_subphase([16, 17, 18, 19], first=True, with_bias=True)

            # ---------- LSTM layer 0 ----------
            wih0_sb = [pw.tile([128, 4 * U], FP, tag="wt", name="lstw_t") for _ in range(12)]
            for k in range(12):
                nc.sync.dma_start(wih0_sb[k][:], wih0T[k * 128:(k + 1) * 128, :])
            whh0_sb = [pw.tile([128, 4 * U], FP, tag="wt", name="lstw_t") for _ in range(KE)]
            for k in range(KE):
                nc.sync.dma_start(whh0_sb[k][:], whh0T[k * 128:(k + 1) * 128, :])

            def lstm_layer(tagp, x_tiles, wih_sb, h_tiles, whh_sb, bias_sb,
                           c_in, c_out, h_out):
                ps_g = ps_m.tile([B, 4 * U], FP, tag="misc", space="PSUM")
                pairs = list(zip(x_tiles, wih_sb)) + list(zip(h_tiles, whh_sb))
                for j, (xt, wt) in enumerate(pairs):
                    nc.tensor.matmul(ps_g[:, :], xt[:], wt[:],
                                     start=(j == 0), stop=False)
                nc.tensor.matmul(ps_g[:, :], ones[:, 0:B], bias_sb[:],
                                 start=False, stop=True)
                gi = pk.tile([B, U], FP, tag="gi")
                gf = pk.tile([B, U], FP, tag="gf")
                gg = pk.tile([B, U], FP, tag="gg")
                go = pk.tile([B, U], FP, tag="go")
                nc.scalar.activation(gi[:], ps_g[:, 0:U], ACTF.Sigmoid)
                nc.scalar.activation(gf[:], ps_g[:, U:2 * U], ACTF.Sigmoid)
                nc.scalar.activation(gg[:], ps_g[:, 2 * U:3 * U], ACTF.Tanh)
                nc.scalar.activation(go[:], ps_g[:, 3 * U:4 * U], ACTF.Sigmoid)
                c_sb = pk.tile([B, U], FP, tag="c_in")
                nc.sync.dma_start(c_sb[:], c_in[:])
                fc_ = pk.tile([B, U], FP, tag="fc_")
                nc.vector.tensor_tensor(out=fc_[:], in0=gf[:], in1=c_sb[:], op=OP.mult)
                ig = pk.tile([B, U], FP, tag="ig")
                nc.vector.tensor_tensor(out=ig[:], in0=gi[:], in1=gg[:], op=OP.mult)
                cn = pk.tile([B, U], FP, tag="cn")
                nc.vector.tensor_tensor(out=cn[:], in0=fc_[:], in1=ig[:], op=OP.add)
                nc.sync.dma_start(c_out[:], cn[:])
                tc_ = pk.tile([B, U], FP, tag="tc_")
                nc.scalar.activation(tc_[:], cn[:], ACTF.Tanh)
                hn = pk.tile([B, U], FP, tag="hn")
                nc.vector.tensor_tensor(out=hn[:], in0=go[:], in1=tc_[:], op=OP.mult)
                nc.sync.dma_start(h_out[:], hn[:])
                # transpose new h -> [U, B] and AllGather to full h.T
                ps_t = ps_m.tile([U, B], FP, tag="misc", space="PSUM")
                nc.tensor.transpose(ps_t[:, :], hn[:], ident[:])
                hnT = pk.tile([U, B], FP, tag="hnT")
                nc.vector.tensor_copy(hnT[:], ps_t[:])
                bnc = dr.tile([U, B], FP)
                nc.sync.dma_start(bnc[:], hnT[:])
                ag = dr.tile([NC * U, B], FP)
                nc.gpsimd.collective_compute(
                    "AllGather", OP.bypass, replica_groups=[list(range(NC))],
                    ins=[bnc.opt()], outs=[ag.opt()],
                )
                newT = [px.tile([128, B], FP, tag=f"{tagp}{k}", name=f"{tagp}{k}") for k in range(KE)]
                for k in range(KE):
                    nc.sync.dma_start(newT[k][:], ag[k * 128:(k + 1) * 128, :])
                return newT

            h0T_new = lstm_layer("n0T", embT_sb + ctxT_sb, wih0_sb, h0T_sb,
                                 whh0_sb, b0_sb, c0s, c0n_o, h0n_o)

            # phase 2: context part of fc
            fc_subphase([8, 9, 10, 11], first=False, with_bias=False)
            fc_subphase([12, 13, 14, 15], first=False, with_bias=False)

            # ---------- LSTM layer 1 ----------
            wih1_sb = [pw.tile([128, 4 * U], FP, tag="wt", name="lstw_t") for _ in range(KE)]
            for k in range(KE):
                nc.sync.dma_start(wih1_sb[k][:], wih1T[k * 128:(k + 1) * 128, :])
            whh1_sb = [pw.tile([128, 4 * U], FP, tag="wt", name="lstw_t") for _ in range(KE)]
            for k in range(KE):
                nc.sync.dma_start(whh1_sb[k][:], whh1T[k * 128:(k + 1) * 128, :])
            h1T_new = lstm_layer("n1T", h0T_new, wih1_sb, h1T_sb, whh1_sb,
                                 b1_sb, c1s, c1n_o, h1n_o)
            for k in range(KE):
                fck_lhs[k] = h1T_new[k]

            # phase 3: h1 part of fc
            fc_subphase([0, 1, 2, 3], first=False, with_bias=False)
            fc_subphase([4, 5, 6, 7], first=False, with_bias=False)

            nc.sync.dma_start(pred[:], pred_acc[:])

    split_multiwait(nc)
    return nc


_BUILT = {}


def _get_nc():
    if "nc" not in _BUILT:
        _BUILT["nc"] = build()
    return _BUILT["nc"]


def _prep_core(m, input_token, hidden, cell, encoder_outputs, mask,
               emb_table, attn_w, attn_b, v_w,
               w_ih0, w_hh0, b_ih0, b_hh0,
               w_ih1, w_hh1, b_ih1, b_hh1, fc_w, fc_b):
    f32 = np.float32
    cm = np.ascontiguousarray
    bsl = slice(m * BL, (m + 1) * BL)
    usl = slice(m * U, (m + 1) * U)

    enc_sh = encoder_outputs[bsl]                       # [BL, S, H2]
    enc_p = cm(enc_sh.transpose(2, 0, 1).reshape(H2, BL * S).astype(f32))

    def lstm_rows(w):
        return np.concatenate([w[g * H2 + m * U:g * H2 + (m + 1) * U]
                               for g in range(4)], axis=0)

    b0 = b_ih0 + b_hh0
    b1 = b_ih1 + b_hh1
    b0_s = np.concatenate([b0[g * H2 + m * U:g * H2 + (m + 1) * U] for g in range(4)])
    b1_s = np.concatenate([b1[g * H2 + m * U:g * H2 + (m + 1) * U] for g in range(4)])
    h1T_full = hidden[1].T.astype(f32)

    return {
        "tokens": cm(input_token.astype(np.int32)),
        "emb_sh": cm(emb_table[:, m * ES:(m + 1) * ES].astype(f32)),
        "h0T": cm(hidden[0].T.astype(f32)),
        "h1T": cm(h1T_full),
        "h1Tl": cm(h1T_full[:, bsl]),
        "c0s": cm(cell[0][:, usl].astype(f32)),
        "c1s": cm(cell[1][:, usl].astype(f32)),
        "enc_p": enc_p,
        "maskf": cm(mask[bsl].astype(f32).reshape(1, BL * S)),
        "attn_wT": cm(attn_w.T.astype(f32)),
        "v_pk": cm(v_w.reshape(MT, 128).T.astype(f32)),
        "ab_pk": cm(attn_b.reshape(MT, 128).T.astype(f32)),
        "wih0T": cm(lstm_rows(w_ih0).T.astype(f32)),
        "whh0T": cm(lstm_rows(w_hh0).T.astype(f32)),
        "wih1T": cm(lstm_rows(w_ih1).T.astype(f32)),
        "whh1T": cm(lstm_rows(w_hh1).T.astype(f32)),
        "bias0": cm(b0_s.reshape(1, 4 * U).astype(f32)),
        "bias1": cm(b1_s.reshape(1, 4 * U).astype(f32)),
        "fcwT": cm(fc_w[m * VS:(m + 1) * VS].T.astype(f32)),
        "fcb": cm(fc_b[m * VS:(m + 1) * VS].reshape(1, VS).astype(f32)),
    }


def kernel(**inputs):
    args = {k: np.asarray(v) for k, v in inputs.items()}
    nc = _get_nc()
    in_maps = [_prep_core(m, **args) for m in range(NC)]
    res = run_bass_kernel_spmd(nc, in_maps, list(range(NC))).results

    prediction = np.concatenate([res[m]["pred"] for m in range(NC)], axis=1)
    h0 = np.concatenate([res[m]["h0n"] for m in range(NC)], axis=1)
    h1 = np.concatenate([res[m]["h1n"] for m in range(NC)], axis=1)
    c0 = np.concatenate([res[m]["c0n"] for m in range(NC)], axis=1)
    c1 = np.concatenate([res[m]["c1n"] for m in range(NC)], axis=1)
    new_hidden = np.stack([h0, h1], axis=0)
    new_cell = np.stack([c0, c1], axis=0)
    attn_weights = np.concatenate([res[m]["attnw"] for m in range(NC)], axis=0)
    return prediction, new_hidden, new_cell, attn_weights
